# revision 8
# baseline (speedup 1.0000x reference)
"""Cross-attention Trainium2 Bass kernel (8 NeuronCores, SPMD, no collectives).

Strategy:
  - Host compacts query rows by mask (masked rows have an exactly uniform
    softmax -> output = mean_m(v) @ Wp + bp, computed on host by linearity).
  - Cores 0-3 handle batch 0's active rows, cores 4-7 batch 1 (context/K/V
    replicated per batch; each core projects kv itself).
  - All matmul operands are bf16 (PE streams 1 cycle/row vs 2 for fp32);
    PSUM accumulation stays fp32.  Scores are computed transposed
    (S^T = K^T-chunks x Q^T, keys on PSUM partitions) into 3-bank PSUM
    tiles of 3 key-chunks so each ACT Exp instruction covers FD=3*nb,
    amortizing the ~300-cycle per-instruction ACT overhead (ACT exp is
    the roofline engine for this kernel).  Softmax denominator comes from
    a ones column appended to V (stationary [128, 33]); normalization via
    DVE reciprocal_approx_fast + PE-broadcast; per-head out-projection
    back to natural [rows, 256] layout.
"""

import math
import os
import sys
import types

import numpy as np

B = 2
N = 8192
M = 2048
D = 256
H = 8
HD = D // H
SCALE = HD ** -0.5

NLOC = 1040          # rows per core (padded)
NB_PER_B = 4 * NLOC  # active-row capacity per batch per launch
BLOCKS = [(0, 384), (384, 384), (768, 272)]
KC = M // 128        # 16 key chunks
GROUPS = [(0, 3), (3, 3), (6, 3), (9, 3), (12, 3), (15, 1)]

_prog = None


def _install_profhook():
    """Make run_bass_kernel_spmd(trace=True) work: this image's antenv lacks
    axon_hooks, so inject it and register trn_boot's ctypes NTFF hook."""
    try:
        if "antenv.axon_hooks" not in sys.modules:
            import antenv
            mod = types.ModuleType("antenv.axon_hooks")
            mod._hook = None
            mod.set_axon_ntff_profile_hook = lambda h: setattr(mod, "_hook", h)
            mod.get_axon_ntff_profile_hook = lambda: mod._hook
            sys.modules["antenv.axon_hooks"] = mod
            antenv.axon_hooks = mod
        from antenv.axon_hooks import (
            get_axon_ntff_profile_hook,
            set_axon_ntff_profile_hook,
        )
        if get_axon_ntff_profile_hook() is None:
            from trn_agent_boot.trn_boot import _ntff_profile_via_ctypes
            so = "/opt/axon/libaxon_pjrt.so"
            if os.path.exists(so):
                set_axon_ntff_profile_hook(_ntff_profile_via_ctypes(so))
    except Exception:
        pass


def _enable_ldw_opt():
    import concourse.bass_utils as bu
    if getattr(bu, "_ldw_opt_patched", False):
        return
    orig = bu.run_command
    def patched(argv, **kw):
        argv = ["--enable-ldw-opt=true" if a == "--enable-ldw-opt=false" else a
                for a in argv]
        return orig(argv, **kw)
    bu.run_command = patched
    bu._ldw_opt_patched = True


def _build_program():
    import concourse.bacc as bacc
    import concourse.mybir as mybir
    import concourse.tile as tile

    f32 = mybir.dt.float32
    f32r = mybir.dt.float32r
    bf16 = mybir.dt.bfloat16
    Exp = mybir.ActivationFunctionType.Exp

    _enable_ldw_opt()
    nc = bacc.Bacc("TRN2", num_devices=8)

    xT = nc.declare_dram_parameter("xT", [D, NLOC], f32r, isOutput=False)
    ctxT = nc.declare_dram_parameter("ctxT", [D, M], f32r, isOutput=False)
    Wq = nc.declare_dram_parameter("Wq", [D, D], f32r, isOutput=False)
    Wkk = nc.declare_dram_parameter("Wkk", [D, D], f32r, isOutput=False)
    Wvv = nc.declare_dram_parameter("Wvv", [D, D], f32r, isOutput=False)
    Wp = nc.declare_dram_parameter("Wp", [D, D], f32r, isOutput=False)
    bqT = nc.declare_dram_parameter("bqT", [1, D], f32, isOutput=False)
    bkkT = nc.declare_dram_parameter("bkkT", [1, D], f32, isOutput=False)
    bvvT = nc.declare_dram_parameter("bvvT", [1, D], f32, isOutput=False)
    bpT = nc.declare_dram_parameter("bpT", [1, D], f32, isOutput=False)
    out = nc.declare_dram_parameter("out", [NLOC, D], f32, isOutput=True)

    with tile.TileContext(nc) as tc:
        with (
            tc.tile_pool(name="w", bufs=1) as wpool,
            tc.tile_pool(name="xc", bufs=4) as xcpool,
            tc.tile_pool(name="acts", bufs=1) as apool,
            tc.tile_pool(name="pt", bufs=4) as ptpool,
            tc.tile_pool(name="otn", bufs=4) as otpool,
            tc.tile_pool(name="small", bufs=4) as spool,
            tc.tile_pool(name="osb", bufs=3) as opool,
            tc.tile_pool(name="ps_sc", bufs=2, space="PSUM") as ps_sc,
            tc.tile_pool(name="ps_att", bufs=2, space="PSUM") as ps_att,
        ):
            # ---- constants / weights to SBUF ----
            ones_col = wpool.tile([1, 128], f32)
            nc.vector.memset(ones_col[:], 1.0)
            ones_row = wpool.tile([1, 512], f32)
            nc.vector.memset(ones_row[:], 1.0)
            onesf = wpool.tile([128, 32], f32)
            nc.vector.memset(onesf[:], 1.0)

            wq_sb = wpool.tile([128, 2, D], f32r)
            wkk_sb = wpool.tile([128, 2, D], f32r)
            wvv_sb = wpool.tile([128, 2, D], f32r)
            for c in range(2):
                nc.sync.dma_start(wq_sb[:, c, :], Wq[128 * c:128 * (c + 1), :])
                nc.sync.dma_start(wkk_sb[:, c, :], Wkk[128 * c:128 * (c + 1), :])
                nc.sync.dma_start(wvv_sb[:, c, :], Wvv[128 * c:128 * (c + 1), :])
            wp2 = wpool.tile([128, 2, D], f32r)
            for c in range(2):
                nc.sync.dma_start(wp2[:, c, :], Wp[128 * c:128 * (c + 1), :])
            bq_sb = wpool.tile([1, D], f32)
            bkk_sb = wpool.tile([1, D], f32)
            bvv_sb = wpool.tile([1, D], f32)
            bp_sb = wpool.tile([1, D], f32)
            nc.sync.dma_start(bq_sb[:], bqT[:])
            nc.sync.dma_start(bkk_sb[:], bkkT[:])
            nc.sync.dma_start(bvv_sb[:], bvvT[:])
            nc.sync.dma_start(bp_sb[:], bpT[:])

            # ---- persistent activations ----
            qT_sb = apool.tile([128, 2, NLOC], f32r)
            kT_sb = apool.tile([128, 2, M], f32r)
            v33 = apool.tile([128, KC, H * 33], f32)
            nc.vector.memset(v33[:], 1.0)

            # k/v projection in one pass over ctxT chunks
            for ms in range(4):
                ccs = []
                for cin in range(2):
                    cc = xcpool.tile([128, 512], f32r, tag="xc", name=f"cc{cin}")
                    nc.sync.dma_start(cc[:], ctxT[128 * cin:128 * (cin + 1), 512 * ms:512 * (ms + 1)])
                    ccs.append(cc)
                # kT[t] chunk = Wkk[:, t].T @ ctx^T chunk (+ bkk)
                for t in range(2):
                    ps = ps_att.tile([128, 512], f32, tag="att", name="psk")
                    for cin in range(2):
                        nc.tensor.matmul(
                            ps[:],
                            wkk_sb[:, cin, 128 * t:128 * (t + 1)],
                            ccs[cin][:],
                            start=(cin == 0), stop=False)
                    nc.tensor.matmul(
                        ps[:],
                        bkk_sb[0:1, 128 * t:128 * (t + 1)],
                        ones_row[0:1, :512],
                        start=False, stop=True)
                    nc.vector.tensor_copy(kT_sb[:, t, 512 * ms:512 * (ms + 1)], ps[:])
                # v chunks (natural layout): mc = 4*ms + i
                for i in range(4):
                    mc = 4 * ms + i
                    ps = ps_att.tile([128, 512], f32, tag="att", name="psv")
                    for cin in range(2):
                        nc.tensor.matmul(
                            ps[:, :D],
                            ccs[cin][:, 128 * i:128 * (i + 1)],
                            wvv_sb[:, cin, :],
                            start=(cin == 0), stop=False)
                    nc.tensor.matmul(
                        ps[:, :D], ones_col[0:1, 0:128], bvv_sb[0:1, :],
                        start=False, stop=True)
                    nc.vector.tensor_copy(
                        v33[:, mc, :].rearrange("p (h w) -> p h w", w=33)[:, :, 0:32],
                        ps[:, :D].rearrange("p (h w) -> p h w", w=32))

            # q projection: qT[t] = Wq[:, t-chunk].T @ x^T (+ bq)
            for off, nb in BLOCKS:
                xcs = []
                for cin in range(2):
                    xc = xcpool.tile([128, 512], f32r, tag="xc", name=f"xc{cin}")
                    nc.sync.dma_start(xc[:, :nb], xT[128 * cin:128 * (cin + 1), off:off + nb])
                    xcs.append(xc)
                for t in range(2):
                    ps = ps_att.tile([128, 512], f32, tag="att", name="psq")
                    for cin in range(2):
                        nc.tensor.matmul(
                            ps[:, :nb],
                            wq_sb[:, cin, 128 * t:128 * (t + 1)],
                            xcs[cin][:, :nb],
                            start=(cin == 0), stop=False)
                    nc.tensor.matmul(
                        ps[:, :nb],
                        bq_sb[0:1, 128 * t:128 * (t + 1)],
                        ones_row[0:1, :nb],
                        start=False, stop=True)
                    nc.vector.tensor_copy(qT_sb[:, t, off:off + nb], ps[:, :nb])

            # ---- attention (software-pipelined over head pairs) ----
            pair_list = []
            for bi, (off, nb) in enumerate(BLOCKS):
                for t in range(2):
                    for p in range(2):
                        pair_list.append((bi, off, nb, t, p))

            otn_by_block = [{} for _ in BLOCKS]
            prev = None  # (bi, off, nb, t, p, hA, hB, ptA, ptB)

            def emit_attnv_kc(po, kc, nb_p, hA_p, hB_p, ptA_p, ptB_p):
                stt, spp = kc == 0, kc == KC - 1
                nc.tensor.matmul(
                    po[0:33, :nb_p], v33[:, kc, 33 * hA_p:33 * hA_p + 33],
                    ptA_p[:, kc, :nb_p], start=stt, stop=spp,
                    tile_position=(0, 0))
                nc.tensor.matmul(
                    po[64:97, :nb_p], v33[:, kc, 33 * hB_p:33 * hB_p + 33],
                    ptB_p[:, kc, :nb_p], start=stt, stop=spp,
                    tile_position=(0, 64))

            def emit_epilogue(po, bi_p, nb_p, t_p, p_p):
                rec128 = spool.tile([128, 384], f32, tag="rec", name="rec128")
                nc.vector.reciprocal_approx_fast(rec128[:, :nb_p], po[:, :nb_p])
                if t_p not in otn_by_block[bi_p]:
                    otn_by_block[bi_p][t_p] = otpool.tile(
                        [128, 384], f32r, tag="otn", name="ot")
                ot = otn_by_block[bi_p][t_p]
                rbase2 = 64 * p_p
                bc = ps_att.tile([128, 512], f32, tag="att", name="bc")
                for obase, lbase, r in ((0, 32, 2 * p_p), (64, 96, 2 * p_p + 1)):
                    nc.tensor.matmul(
                        bc[32 * r:32 * r + 32, :nb_p],
                        onesf[lbase:lbase + 1, 0:32],
                        rec128[lbase:lbase + 1, :nb_p],
                        start=True, stop=True, tile_position=(lbase, 32 * r))
                    nc.vector.tensor_copy(
                        ot[32 * r:32 * r + 32, :nb_p], po[obase:obase + 32, :nb_p])
                nc.vector.tensor_mul(
                    ot[rbase2:rbase2 + 64, :nb_p],
                    ot[rbase2:rbase2 + 64, :nb_p],
                    bc[rbase2:rbase2 + 64, :nb_p])

            def emit_outproj(bi_p):
                off_p, nb_p = BLOCKS[bi_p]
                otn_t = otn_by_block[bi_p]
                qsizes = []
                q0 = 0
                while q0 < nb_p:
                    qsizes.append((q0, min(128, nb_p - q0)))
                    q0 += 128
                for q0, qn in qsizes:
                    pso = ps_att.tile([128, 512], f32, tag="att", name="pso")
                    for t_ in range(2):
                        nc.tensor.matmul(
                            pso[0:qn, 0:D],
                            otn_t[t_][:, q0:q0 + qn],
                            wp2[:, t_, :],
                            start=(t_ == 0), stop=False)
                    nc.tensor.matmul(
                        pso[0:qn, 0:D], ones_col[0:1, 0:qn], bp_sb[0:1, :],
                        start=False, stop=True)
                    ob = opool.tile([128, D], f32, tag="ob", name="ob")
                    nc.vector.tensor_copy(ob[0:qn, :], pso[0:qn, 0:D])
                    nc.sync.dma_start(out[off_p + q0:off_p + q0 + qn, :], ob[0:qn, :])

            for i in range(len(pair_list) + 1):
                cur = pair_list[i] if i < len(pair_list) else None
                po_prev = None
                if prev is not None:
                    po_prev = ps_att.tile([128, 512], f32, tag="att", name="po")
                    bi_p, off_p, nb_p, t_p, p_p, hA_p, hB_p, ptA_p, ptB_p = prev
                if cur is not None:
                    bi, off, nb, t, p = cur
                    rA, rB = 2 * p, 2 * p + 1
                    hA, hB = 4 * t + rA, 4 * t + rB
                    ptA = ptpool.tile([128, KC, 384], f32, tag="pt", name="ptA")
                    ptB = ptpool.tile([128, KC, 384], f32, tag="pt", name="ptB")
                    for kc0, glen in GROUPS:
                        psA = ps_sc.tile([128, 3, 512], f32, tag="sc", name="psA")
                        psB = ps_sc.tile([128, 3, 512], f32, tag="sc", name="psB")
                        for j in range(glen):
                            kc = kc0 + j
                            nc.tensor.matmul(
                                psA[:, j, :nb],
                                kT_sb[32 * rA:32 * rA + 32, t, 128 * kc:128 * (kc + 1)],
                                qT_sb[32 * rA:32 * rA + 32, t, off:off + nb],
                                start=True, stop=True,
                                tile_position=(32 * rA, 0))
                            nc.tensor.matmul(
                                psB[:, j, :nb],
                                kT_sb[32 * rB:32 * rB + 32, t, 128 * kc:128 * (kc + 1)],
                                qT_sb[32 * rB:32 * rB + 32, t, off:off + nb],
                                start=True, stop=True,
                                tile_position=(32 * rB, 0))
                        nc.scalar.activation(
                            ptA[:, kc0:kc0 + glen, :nb],
                            psA[:, 0:glen, :nb], Exp, scale=SCALE)
                        nc.scalar.activation(
                            ptB[:, kc0:kc0 + glen, :nb],
                            psB[:, 0:glen, :nb], Exp, scale=SCALE)
                        if prev is not None:
                            for j in range(glen):
                                emit_attnv_kc(po_prev, kc0 + j, nb_p,
                                              hA_p, hB_p, ptA_p, ptB_p)
                else:
                    for kc in range(KC):
                        emit_attnv_kc(po_prev, kc, nb_p, hA_p, hB_p, ptA_p, ptB_p)
                if prev is not None:
                    emit_epilogue(po_prev, bi_p, nb_p, t_p, p_p)
                    if t_p == 1 and p_p == 1:
                        emit_outproj(bi_p)
                if cur is not None:
                    prev = (bi, off, nb, t, p, hA, hB, ptA, ptB)

    nc.compile()
    return nc


def _get_program():
    global _prog
    if _prog is None:
        _prog = _build_program()
    return _prog


def kernel(x, context, mask, Wq, bq, Wkv, bkv, Wp, bp):
    from concourse.bass_utils import run_bass_kernel_spmd

    profile = bool(int(os.environ.get("BASS_KERNEL_PROFILE", "0")))
    if profile:
        _install_profhook()

    x = np.ascontiguousarray(np.asarray(x, dtype=np.float32))
    context = np.ascontiguousarray(np.asarray(context, dtype=np.float32))
    mask = np.asarray(mask).astype(bool)
    Wq = np.asarray(Wq, dtype=np.float32)
    bq = np.asarray(bq, dtype=np.float32)
    Wkv = np.asarray(Wkv, dtype=np.float32)
    bkv = np.asarray(bkv, dtype=np.float32)
    Wp = np.asarray(Wp, dtype=np.float32)
    bp = np.asarray(bp, dtype=np.float32)

    nc = _get_program()

    out = np.empty((B, N, D), dtype=np.float32)
    # Masked rows: softmax over a constant row is exactly uniform ->
    # attn output = mean_m(v) = mean_m(context) @ Wkv_v + bkv_v (linearity).
    for b in range(B):
        vm = context[b].mean(axis=0) @ Wkv[:, D:] + bkv[D:]
        out[b][~mask[b]] = vm @ Wp + bp

    idx = [np.flatnonzero(mask[b]) for b in range(B)]
    n_launch = max(1, *(int(math.ceil(len(i) / NB_PER_B)) for i in idx))

    weights = {
        "Wq": Wq, "Wkk": np.ascontiguousarray(Wkv[:, :D]),
        "Wvv": np.ascontiguousarray(Wkv[:, D:]), "Wp": Wp,
        "bqT": bq.reshape(1, D),
        "bkkT": np.ascontiguousarray(bkv[:D]).reshape(1, D),
        "bvvT": np.ascontiguousarray(bkv[D:]).reshape(1, D),
        "bpT": bp.reshape(1, D),
    }
    xb = [x[b] for b in range(B)]
    ctxT = [np.ascontiguousarray(context[b].T) for b in range(B)]

    prof_ns = []
    for li in range(n_launch):
        in_maps = []
        rowsets = []
        for core in range(8):
            b = core // 4
            lo = li * NB_PER_B + (core % 4) * NLOC
            rows = idx[b][lo:lo + NLOC]
            rowsets.append((b, rows))
            xTc = np.zeros((D, NLOC), dtype=np.float32)
            if len(rows):
                xTc[:, :len(rows)] = xb[b][rows].T
            in_maps.append({"xT": xTc, "ctxT": ctxT[b], **weights})
        res = run_bass_kernel_spmd(nc, in_maps, list(range(8)), trace=profile)
        if profile and res.exec_time_ns is not None:
            prof_ns.append(res)
        for core in range(8):
            b, rows = rowsets[core]
            if len(rows):
                out[b][rows] = res.results[core]["out"][:len(rows)]

    if profile and prof_ns:
        kernel.last_results = prof_ns
        kernel.last_exec_ns = max(r.exec_time_ns for r in prof_ns)
    return out


# revision 10
# speedup vs baseline: 1.2738x; 1.2738x over previous
"""Cross-attention Trainium2 Bass kernel (8 NeuronCores, SPMD, no collectives).

Strategy:
  - Host compacts query rows by mask (masked rows have an exactly uniform
    softmax -> output = mean_m(v) @ Wp + bp, computed on host by linearity).
  - Cores 0-3 handle batch 0's active rows, cores 4-7 batch 1 (context/K/V
    replicated per batch; each core projects kv itself).
  - Matmul dtypes: fp32r for scores/projections (1.5 cyc/row and compatible
    with --enable-ldw-opt so weight loads overlap matmuls), plain fp32 for
    attn@v (fp32r cannot write PSUM partition base 64, which the col-tiled
    second head needs).  PSUM accumulation is always fp32.
  - Scores are computed transposed (S^T = K^T-chunks x Q^T, keys on PSUM
    partitions) into 3-bank PSUM tiles of 3 key-chunks so each ACT Exp
    instruction covers FD=3*nb, amortizing the ~300-cycle per-instruction
    ACT overhead (ACT exp is the roofline engine for this kernel).
  - All biases are folded away from the hot path: k/q bias is fused into the
    PSUM->SBUF copy as a per-partition tensor_scalar_add; v bias and the
    out-projection bias commute through softmax (weights sum to 1) and are
    added on the host as bp_eff = bvv @ Wp + bp.
  - kv/q projection work is interleaved into the attention pair loop so the
    scalar engine starts exp work as early as possible.
  - Softmax denominator via a ones column appended to V (stationary
    [128, 33]); normalization by DVE reciprocal_approx_fast + PE-broadcast;
    per-head out-projection back to natural [rows, 256] layout.
"""

import math
import os
import sys
import types

import numpy as np

B = 2
N = 8192
M = 2048
D = 256
H = 8
HD = D // H
SCALE = HD ** -0.5

NLOC = 1040          # rows per core (padded)
NB_PER_B = 4 * NLOC  # active-row capacity per batch per launch
BLOCKS = [(0, 384), (384, 384), (768, 272)]
KC = M // 128        # 16 key chunks
GROUPS = [(0, 3), (3, 3), (6, 3), (9, 3), (12, 3), (15, 1)]

_prog = None


def _install_profhook():
    """Make run_bass_kernel_spmd(trace=True) work: this image's antenv lacks
    axon_hooks, so inject it and register trn_boot's ctypes NTFF hook."""
    try:
        if "antenv.axon_hooks" not in sys.modules:
            import antenv
            mod = types.ModuleType("antenv.axon_hooks")
            mod._hook = None
            mod.set_axon_ntff_profile_hook = lambda h: setattr(mod, "_hook", h)
            mod.get_axon_ntff_profile_hook = lambda: mod._hook
            sys.modules["antenv.axon_hooks"] = mod
            antenv.axon_hooks = mod
        from antenv.axon_hooks import (
            get_axon_ntff_profile_hook,
            set_axon_ntff_profile_hook,
        )
        if get_axon_ntff_profile_hook() is None:
            from trn_agent_boot.trn_boot import _ntff_profile_via_ctypes
            so = "/opt/axon/libaxon_pjrt.so"
            if os.path.exists(so):
                set_axon_ntff_profile_hook(_ntff_profile_via_ctypes(so))
    except Exception:
        pass


def _enable_ldw_opt():
    import concourse.bass_utils as bu
    if getattr(bu, "_ldw_opt_patched", False):
        return
    orig = bu.run_command
    def patched(argv, **kw):
        argv = ["--enable-ldw-opt=true" if a == "--enable-ldw-opt=false" else a
                for a in argv]
        return orig(argv, **kw)
    bu.run_command = patched
    bu._ldw_opt_patched = True


def _build_program():
    import concourse.bacc as bacc
    import concourse.mybir as mybir
    import concourse.tile as tile

    f32 = mybir.dt.float32
    f32r = mybir.dt.float32r
    Exp = mybir.ActivationFunctionType.Exp

    _enable_ldw_opt()
    nc = bacc.Bacc("TRN2", num_devices=8)

    xT = nc.declare_dram_parameter("xT", [D, NLOC], f32r, isOutput=False)
    ctxT = nc.declare_dram_parameter("ctxT", [D, M], f32r, isOutput=False)
    Wq = nc.declare_dram_parameter("Wq", [D, D], f32r, isOutput=False)
    Wkk = nc.declare_dram_parameter("Wkk", [D, D], f32r, isOutput=False)
    Wvv = nc.declare_dram_parameter("Wvv", [D, D], f32r, isOutput=False)
    Wp = nc.declare_dram_parameter("Wp", [D, D], f32r, isOutput=False)
    bqC = nc.declare_dram_parameter("bqC", [128, 2], f32, isOutput=False)
    bkkC = nc.declare_dram_parameter("bkkC", [128, 2], f32, isOutput=False)
    out = nc.declare_dram_parameter("out", [NLOC, D], f32, isOutput=True)

    with tile.TileContext(nc) as tc:
        with (
            tc.tile_pool(name="w", bufs=1) as wpool,
            tc.tile_pool(name="xc", bufs=4) as xcpool,
            tc.tile_pool(name="acts", bufs=1) as apool,
            tc.tile_pool(name="pt", bufs=4) as ptpool,
            tc.tile_pool(name="otn", bufs=4) as otpool,
            tc.tile_pool(name="small", bufs=4) as spool,
            tc.tile_pool(name="osb", bufs=3) as opool,
            tc.tile_pool(name="ps_sc", bufs=2, space="PSUM") as ps_sc,
            tc.tile_pool(name="ps_att", bufs=2, space="PSUM") as ps_att,
        ):
            # ---- constants / weights to SBUF ----
            onesf = wpool.tile([128, 32], f32)
            nc.vector.memset(onesf[:], 1.0)

            wq_sb = wpool.tile([128, 2, D], f32r)
            wkk_sb = wpool.tile([128, 2, D], f32r)
            wvv_sb = wpool.tile([128, 2, D], f32r)
            for c in range(2):
                nc.sync.dma_start(wq_sb[:, c, :], Wq[128 * c:128 * (c + 1), :])
                nc.sync.dma_start(wkk_sb[:, c, :], Wkk[128 * c:128 * (c + 1), :])
                nc.sync.dma_start(wvv_sb[:, c, :], Wvv[128 * c:128 * (c + 1), :])
            wp2 = wpool.tile([128, 2, D], f32r)
            for c in range(2):
                nc.sync.dma_start(wp2[:, c, :], Wp[128 * c:128 * (c + 1), :])
            bq_sb = wpool.tile([128, 2], f32)
            bkk_sb = wpool.tile([128, 2], f32)
            nc.sync.dma_start(bq_sb[:], bqC[:])
            nc.sync.dma_start(bkk_sb[:], bkkC[:])

            # ---- persistent activations ----
            qT_sb = apool.tile([128, 2, NLOC], f32r)
            kT_sb = apool.tile([128, 2, M], f32r)
            v33 = apool.tile([128, KC, H * 33], f32)
            nc.vector.memset(v33[:], 1.0)

            def emit_kv_ms(ms):
                """kv projection for context chunk ms (512 keys)."""
                ccs = []
                for cin in range(2):
                    cc = xcpool.tile([128, 512], f32r, tag="xc", name=f"cc{cin}")
                    nc.sync.dma_start(cc[:], ctxT[128 * cin:128 * (cin + 1), 512 * ms:512 * (ms + 1)])
                    ccs.append(cc)
                # kT[t] chunk = Wkk[:, t].T @ ctx^T chunk; bias fused into copy
                for t in range(2):
                    ps = ps_att.tile([128, 512], f32, tag="att", name="psk")
                    for cin in range(2):
                        nc.tensor.matmul(
                            ps[:],
                            wkk_sb[:, cin, 128 * t:128 * (t + 1)],
                            ccs[cin][:],
                            start=(cin == 0), stop=(cin == 1))
                    nc.vector.tensor_scalar_add(
                        kT_sb[:, t, 512 * ms:512 * (ms + 1)], ps[:],
                        bkk_sb[:, t:t + 1])
                # v chunks (natural layout): mc = 4*ms + i; v-bias folded to host
                for i in range(4):
                    mc = 4 * ms + i
                    ps = ps_att.tile([128, 512], f32, tag="att", name="psv")
                    for cin in range(2):
                        nc.tensor.matmul(
                            ps[:, :D],
                            ccs[cin][:, 128 * i:128 * (i + 1)],
                            wvv_sb[:, cin, :],
                            start=(cin == 0), stop=(cin == 1))
                    nc.vector.tensor_copy(
                        v33[:, mc, :].rearrange("p (h w) -> p h w", w=33)[:, :, 0:32],
                        ps[:, :D].rearrange("p (h w) -> p h w", w=32))

            def emit_qproj(bi):
                off, nb = BLOCKS[bi]
                xcs = []
                for cin in range(2):
                    xc = xcpool.tile([128, 512], f32r, tag="xc", name=f"xc{cin}")
                    nc.sync.dma_start(xc[:, :nb], xT[128 * cin:128 * (cin + 1), off:off + nb])
                    xcs.append(xc)
                for t in range(2):
                    ps = ps_att.tile([128, 512], f32, tag="att", name="psq")
                    for cin in range(2):
                        nc.tensor.matmul(
                            ps[:, :nb],
                            wq_sb[:, cin, 128 * t:128 * (t + 1)],
                            xcs[cin][:, :nb],
                            start=(cin == 0), stop=(cin == 1))
                    nc.vector.tensor_scalar_add(
                        qT_sb[:, t, off:off + nb], ps[:, :nb], bq_sb[:, t:t + 1])

            # startup: only what pair 0 needs immediately; the rest is
            # interleaved into the pair loop below.
            emit_kv_ms(0)
            emit_qproj(0)
            tail_work = [lambda: emit_kv_ms(1), lambda: emit_kv_ms(2),
                         lambda: emit_kv_ms(3), lambda: emit_qproj(1),
                         lambda: emit_qproj(2)]

            # ---- attention (software-pipelined over head pairs) ----
            pair_list = []
            for bi, (off, nb) in enumerate(BLOCKS):
                for t in range(2):
                    for p in range(2):
                        pair_list.append((bi, off, nb, t, p))

            otn_by_block = [{} for _ in BLOCKS]
            prev = None  # (bi, off, nb, t, p, hA, hB, ptA, ptB)

            def emit_attnv_kc(po, kc, nb_p, hA_p, hB_p, ptA_p, ptB_p):
                stt, spp = kc == 0, kc == KC - 1
                nc.tensor.matmul(
                    po[0:33, :nb_p], v33[:, kc, 33 * hA_p:33 * hA_p + 33],
                    ptA_p[:, kc, :nb_p], start=stt, stop=spp,
                    tile_position=(0, 0))
                nc.tensor.matmul(
                    po[64:97, :nb_p], v33[:, kc, 33 * hB_p:33 * hB_p + 33],
                    ptB_p[:, kc, :nb_p], start=stt, stop=spp,
                    tile_position=(0, 64))

            def emit_epilogue(po, bi_p, nb_p, t_p, p_p):
                rec128 = spool.tile([128, 384], f32, tag="rec", name="rec128")
                nc.vector.reciprocal_approx_fast(rec128[:, :nb_p], po[:, :nb_p])
                if t_p not in otn_by_block[bi_p]:
                    otn_by_block[bi_p][t_p] = otpool.tile(
                        [128, 384], f32r, tag="otn", name="ot")
                ot = otn_by_block[bi_p][t_p]
                rbase2 = 64 * p_p
                bc = ps_att.tile([128, 512], f32, tag="att", name="bc")
                for obase, lbase, r in ((0, 32, 2 * p_p), (64, 96, 2 * p_p + 1)):
                    nc.tensor.matmul(
                        bc[32 * r:32 * r + 32, :nb_p],
                        onesf[lbase:lbase + 1, 0:32],
                        rec128[lbase:lbase + 1, :nb_p],
                        start=True, stop=True, tile_position=(lbase, 32 * r))
                    nc.vector.tensor_copy(
                        ot[32 * r:32 * r + 32, :nb_p], po[obase:obase + 32, :nb_p])
                nc.vector.tensor_mul(
                    ot[rbase2:rbase2 + 64, :nb_p],
                    ot[rbase2:rbase2 + 64, :nb_p],
                    bc[rbase2:rbase2 + 64, :nb_p])

            def emit_outproj(bi_p):
                off_p, nb_p = BLOCKS[bi_p]
                otn_t = otn_by_block[bi_p]
                qsizes = []
                q0 = 0
                while q0 < nb_p:
                    qsizes.append((q0, min(128, nb_p - q0)))
                    q0 += 128
                for q0, qn in qsizes:
                    pso = ps_att.tile([128, 512], f32, tag="att", name="pso")
                    for t_ in range(2):
                        nc.tensor.matmul(
                            pso[0:qn, 0:D],
                            otn_t[t_][:, q0:q0 + qn],
                            wp2[:, t_, :],
                            start=(t_ == 0), stop=(t_ == 1))
                    ob = opool.tile([128, D], f32, tag="ob", name="ob")
                    nc.vector.tensor_copy(ob[0:qn, :], pso[0:qn, 0:D])
                    nc.sync.dma_start(out[off_p + q0:off_p + q0 + qn, :], ob[0:qn, :])

            for i in range(len(pair_list) + 1):
                cur = pair_list[i] if i < len(pair_list) else None
                po_prev = None
                if prev is not None:
                    po_prev = ps_att.tile([128, 512], f32, tag="att", name="po")
                    bi_p, off_p, nb_p, t_p, p_p, hA_p, hB_p, ptA_p, ptB_p = prev
                if cur is not None:
                    bi, off, nb, t, p = cur
                    rA, rB = 2 * p, 2 * p + 1
                    hA, hB = 4 * t + rA, 4 * t + rB
                    ptA = ptpool.tile([128, KC, 384], f32, tag="pt", name="ptA")
                    ptB = ptpool.tile([128, KC, 384], f32, tag="pt", name="ptB")
                    for kc0, glen in GROUPS:
                        psA = ps_sc.tile([128, 3, 512], f32, tag="sc", name="psA")
                        psB = ps_sc.tile([128, 3, 512], f32, tag="sc", name="psB")
                        for j in range(glen):
                            kc = kc0 + j
                            nc.tensor.matmul(
                                psA[:, j, :nb],
                                kT_sb[32 * rA:32 * rA + 32, t, 128 * kc:128 * (kc + 1)],
                                qT_sb[32 * rA:32 * rA + 32, t, off:off + nb],
                                start=True, stop=True,
                                tile_position=(32 * rA, 0))
                            nc.tensor.matmul(
                                psB[:, j, :nb],
                                kT_sb[32 * rB:32 * rB + 32, t, 128 * kc:128 * (kc + 1)],
                                qT_sb[32 * rB:32 * rB + 32, t, off:off + nb],
                                start=True, stop=True,
                                tile_position=(32 * rB, 0))
                        nc.scalar.activation(
                            ptA[:, kc0:kc0 + glen, :nb],
                            psA[:, 0:glen, :nb], Exp, scale=SCALE)
                        nc.scalar.activation(
                            ptB[:, kc0:kc0 + glen, :nb],
                            psB[:, 0:glen, :nb], Exp, scale=SCALE)
                        if prev is not None:
                            for j in range(glen):
                                emit_attnv_kc(po_prev, kc0 + j, nb_p,
                                              hA_p, hB_p, ptA_p, ptB_p)
                        if tail_work:
                            tail_work.pop(0)()
                else:
                    for kc in range(KC):
                        emit_attnv_kc(po_prev, kc, nb_p, hA_p, hB_p, ptA_p, ptB_p)
                if prev is not None:
                    emit_epilogue(po_prev, bi_p, nb_p, t_p, p_p)
                    if t_p == 1 and p_p == 1:
                        emit_outproj(bi_p)
                if cur is not None:
                    prev = (bi, off, nb, t, p, hA, hB, ptA, ptB)

    nc.compile()
    return nc


def _get_program():
    global _prog
    if _prog is None:
        _prog = _build_program()
    return _prog


def kernel(x, context, mask, Wq, bq, Wkv, bkv, Wp, bp):
    from concourse.bass_utils import run_bass_kernel_spmd

    profile = bool(int(os.environ.get("BASS_KERNEL_PROFILE", "0")))
    if profile:
        _install_profhook()

    x = np.ascontiguousarray(np.asarray(x, dtype=np.float32))
    context = np.ascontiguousarray(np.asarray(context, dtype=np.float32))
    mask = np.asarray(mask).astype(bool)
    Wq = np.asarray(Wq, dtype=np.float32)
    bq = np.asarray(bq, dtype=np.float32)
    Wkv = np.asarray(Wkv, dtype=np.float32)
    bkv = np.asarray(bkv, dtype=np.float32)
    Wp = np.asarray(Wp, dtype=np.float32)
    bp = np.asarray(bp, dtype=np.float32)

    nc = _get_program()

    out = np.empty((B, N, D), dtype=np.float32)
    # Masked rows: softmax over a constant row is exactly uniform ->
    # attn output = mean_m(v) = mean_m(context) @ Wkv_v + bkv_v (linearity).
    for b in range(B):
        vm = context[b].mean(axis=0) @ Wkv[:, D:] + bkv[D:]
        out[b][~mask[b]] = vm @ Wp + bp

    # Device computes attention with V un-biased and no out-proj bias;
    # both commute through softmax (weights sum to 1): add on host.
    bp_eff = (bkv[D:] @ Wp + bp).astype(np.float32)

    idx = [np.flatnonzero(mask[b]) for b in range(B)]
    n_launch = max(1, *(int(math.ceil(len(i) / NB_PER_B)) for i in idx))

    weights = {
        "Wq": Wq, "Wkk": np.ascontiguousarray(Wkv[:, :D]),
        "Wvv": np.ascontiguousarray(Wkv[:, D:]), "Wp": Wp,
        "bqC": np.ascontiguousarray(bq.reshape(2, 128).T),
        "bkkC": np.ascontiguousarray(bkv[:D].reshape(2, 128).T),
    }
    xb = [x[b] for b in range(B)]
    ctxT = [np.ascontiguousarray(context[b].T) for b in range(B)]

    prof_ns = []
    for li in range(n_launch):
        in_maps = []
        rowsets = []
        for core in range(8):
            b = core // 4
            lo = li * NB_PER_B + (core % 4) * NLOC
            rows = idx[b][lo:lo + NLOC]
            rowsets.append((b, rows))
            xTc = np.zeros((D, NLOC), dtype=np.float32)
            if len(rows):
                xTc[:, :len(rows)] = xb[b][rows].T
            in_maps.append({"xT": xTc, "ctxT": ctxT[b], **weights})
        res = run_bass_kernel_spmd(nc, in_maps, list(range(8)), trace=profile)
        if profile and res.exec_time_ns is not None:
            prof_ns.append(res)
        for core in range(8):
            b, rows = rowsets[core]
            if len(rows):
                out[b][rows] = res.results[core]["out"][:len(rows)] + bp_eff

    if profile and prof_ns:
        kernel.last_results = prof_ns
        kernel.last_exec_ns = max(r.exec_time_ns for r in prof_ns)
    return out
